# revision 1
# baseline (speedup 1.0000x reference)
"""DCNv2 (modulated deformable conv) forward on 8 Trainium2 NeuronCores.

Problem: input [4,64,96,96], offset [4,18,96,96], mask [4,9,96,96],
weight [64,64,3,3], bias [64] -> out [4,64,96,96]. STRIDE=1, PAD=1, DIL=1,
deformable groups G=1.

Sharding: data-parallel over (batch, H-half): core = b*2 + h handles output
rows [48h, 48h+48) of batch b. Each core receives the full image of its
batch (sampling positions are unbounded so no halo margin is safe), plus
its slice of offset/mask; weight/bias replicated.

Per-core device algorithm (all compute on device):
  1. Transpose image [64, 9216] -> pixel-major img2d [9218, 64] in DRAM
     (rows 0 and 9217 are zero pads so clamped gathers read finite data).
  1b. The pixel-major image is written twice into a quad-packed padded
     image imgPad[r] = [pixel(r-97) | pixel(r-1)], so ONE contiguous
     256-f32 read at row r = 96*fy+fx+97 returns all four bilinear
     neighbors [v00, v10, v01, v11] of a sample.
  2. Transpose offset/mask per 128-pixel tile, compute sampling positions,
     floor/frac (cast-roundtrip floor, robust to any rounding mode),
     validity masks, bilinear*mask weights wq, and clamped quad-gather
     indices for the 9 taps.
  3. Per (tile, tap): one indirect DMA (1 row-index per dest partition --
     the only indirect form real HW supports; the multi-index forms and
     the extended Q7 dma_gather/ap_gather ucode are broken under this
     terminal) gathers 128px x 256 f32. Vector engine applies bilinear
     weights + reduces the 4 neighbors -> im2col block samp [128px, 9*64],
     PE transposes it (chunk offsets 0,128,256,384,448 with dedup in the
     weight layout) and contracts with the rearranged weight into
     out [64co, 128px]; bias added from PSUM; HWDGE stores.

Measured on 8 axon trn2 cores: rel err 2.9e-6 vs fp32 reference,
HW exec ~600 us (Pool/SWDGE-bound: 324 indirect DMAs x ~1.1 us SWDGE
generation + ~310 ns issue gap + ring-wrap stalls; ~54 us cold prefix).
"""

import os
import sys
import types
import numpy as np

for _p in ("/opt/trn_rl_repo",):
    if _p not in sys.path and os.path.isdir(_p):
        sys.path.append(_p)

# If the image lacks antenv.axon_hooks, provide a null shim so an externally
# forced BASS_TRACE doesn't crash run_bass_kernel_spmd (it degrades to no
# tracing).
try:
    import antenv.axon_hooks  # noqa: F401
except ImportError:
    _hookmod = types.ModuleType("antenv.axon_hooks")
    _hookmod._hook = None
    _hookmod.set_axon_ntff_profile_hook = lambda h: setattr(_hookmod, "_hook", h)
    _hookmod.get_axon_ntff_profile_hook = lambda: _hookmod._hook
    sys.modules["antenv.axon_hooks"] = _hookmod

B, C, H, W = 4, 64, 96, 96
K = 9
Co = 64
HW = H * W                  # 9216
N_CORES = 8
HHALF = 48
NPIX = HHALF * W            # 4608 output pixels per core
NT = NPIX // 128            # 36 tiles
CHUNK_OFFS = (0, 128, 256, 384, 448)  # samp free-dim transpose chunks

_CACHE = {}


def _build_module():
    from contextlib import ExitStack

    import concourse.bass as bass
    import concourse.tile as tile
    from concourse import bacc, mybir
    from concourse.bass_interp import get_hw_module
    from concourse.masks import make_identity

    f32 = mybir.dt.float32
    i32 = mybir.dt.int32
    Alu = mybir.AluOpType
    Act = mybir.ActivationFunctionType

    nc = bacc.Bacc("TRN2", target_bir_lowering=False, debug=False,
                   enable_asserts=False, num_devices=N_CORES)

    img_ap = nc.dram_tensor("img", [C, HW], f32, kind="ExternalInput").ap()
    offmask_ap = nc.dram_tensor("offmask", [27, NPIX], f32, kind="ExternalInput").ap()
    byx_ap = nc.dram_tensor("byx", [128, NT * K * 2], f32, kind="ExternalInput").ap()
    w2_ap = nc.dram_tensor("w2", [5 * 128, Co], f32, kind="ExternalInput").ap()
    bias_ap = nc.dram_tensor("biasv", [Co, 1], f32, kind="ExternalInput").ap()
    out_ap = nc.dram_tensor("out", [Co, NPIX], f32, kind="ExternalOutput").ap()
    # imgPad[r, 0:64] = pixel(r-97), imgPad[r, 64:128] = pixel(r-1); zero pads.
    # One gathered 256-f32 block at row r=q00+97 yields [v00, v10, v01, v11].
    img_pad_ap = nc.dram_tensor("imgpad", [HW + 98, 2 * C], f32).ap()

    with tile.TileContext(nc) as tc:
        with ExitStack() as ctx:
            cpool = ctx.enter_context(tc.tile_pool(name="consts", bufs=1))
            apool = ctx.enter_context(tc.tile_pool(name="phase_a", bufs=1))
            prep_ctx = ExitStack()
            prep = prep_ctx.enter_context(tc.tile_pool(name="prep", bufs=1))
            tp_ps = ctx.enter_context(tc.tile_pool(name="tr_ps", bufs=4, space="PSUM"))
            tb_ps = ctx.enter_context(tc.tile_pool(name="trB_ps", bufs=2, space="PSUM"))
            opsum = ctx.enter_context(tc.tile_pool(name="opsum", bufs=2, space="PSUM"))

            # ---- constants ----
            ident = cpool.tile([128, 128], f32)
            make_identity(nc, ident[:])
            # offset/mask load first: it gates the A2->A3->gather chain
            om = prep.tile([27, NPIX], f32)
            nc.sync.dma_start(out=om[:], in_=offmask_ap)
            byx_sb = cpool.tile([128, NT * K * 2], f32)
            nc.sync.dma_start(out=byx_sb[:], in_=byx_ap)

            zt = cpool.tile([128, C], f32)
            nc.vector.memset(zt[:], 0.0)
            nc.sync.dma_start(out=img_pad_ap[0:97, 0:C], in_=zt[0:97, :])
            nc.sync.dma_start(out=img_pad_ap[0:1, C:2 * C], in_=zt[0:1, :])
            nc.sync.dma_start(out=img_pad_ap[HW + 97:HW + 98, 0:C], in_=zt[0:1, :])
            nc.sync.dma_start(out=img_pad_ap[HW + 1:HW + 98, C:2 * C], in_=zt[0:97, :])

            w2_sb = cpool.tile([128, 5 * Co], f32)
            nc.sync.dma_start(
                out=w2_sb[:].rearrange("p (f c) -> p f c", f=5),
                in_=w2_ap.rearrange("(f p) c -> p f c", p=128),
            )
            bias_sb = cpool.tile([Co, 1], f32)
            nc.sync.dma_start(out=bias_sb[:], in_=bias_ap)

            # ---- phase A2: offset/mask tile transposes ----
            omT = prep.tile([128, NT * 27], f32)
            for t in range(NT):
                pt = tp_ps.tile([128, 27], f32, tag="tr")
                nc.tensor.transpose(
                    out=pt[:], in_=om[:, t * 128:(t + 1) * 128],
                    identity=ident[:27, :27])
                nc.scalar.activation(
                    out=omT[:, t * 27:(t + 1) * 27], in_=pt[:], func=Act.Copy)

            # ---- phase A1: image -> pixel-major quad-padded imgPad, written
            # in chunks so DMA overlaps the PE transposes ----
            s_img = prep.tile([C, HW], f32)
            for g0 in range(0, 72, 12):
                nc.sync.dma_start(out=s_img[:, g0 * 128:(g0 + 12) * 128],
                                  in_=img_ap[:, g0 * 128:(g0 + 12) * 128])
            s_imgT = prep.tile([128, 72 * C], f32)
            CH = 12
            for g0 in range(0, 72, CH):
                for j in range(g0, g0 + CH):
                    pt = tp_ps.tile([128, C], f32, tag="tr")
                    nc.tensor.transpose(
                        out=pt[:], in_=s_img[:, j * 128:(j + 1) * 128],
                        identity=ident[:C, :C])
                    if j % 2 == 0:
                        nc.scalar.activation(
                            out=s_imgT[:, j * C:(j + 1) * C], in_=pt[:],
                            func=Act.Copy)
                    else:
                        nc.vector.tensor_copy(
                            out=s_imgT[:, j * C:(j + 1) * C], in_=pt[:])
                r0 = g0 * 128
                r1 = (g0 + CH) * 128
                seg = s_imgT[:, g0 * C:(g0 + CH) * C].rearrange(
                    "p (j c) -> p j c", j=CH)
                nc.sync.dma_start(
                    out=img_pad_ap[97 + r0:97 + r1, 0:C].rearrange(
                        "(j p) c -> p j c", p=128),
                    in_=seg)
                nc.sync.dma_start(
                    out=img_pad_ap[1 + r0:1 + r1, C:2 * C].rearrange(
                        "(j p) c -> p j c", p=128),
                    in_=seg)

            # ---- phase A3: index & weight math (batched over all tiles) ----
            omT3 = omT[:].rearrange("p (t c) -> p t c", t=NT)
            dyv = omT3[:, :, 0:18:2]          # [128, NT, 9]
            dxv = omT3[:, :, 1:18:2]
            mv = omT3[:, :, 18:27]
            byx4 = byx_sb[:].rearrange("p (t k s) -> p t k s", t=NT, k=K)
            hov = byx4[:, :, :, 0]          # ho - 1 + ky  [128, NT, K]
            wov = byx4[:, :, :, 1]          # wo - 1 + kx

            def t3(name):
                t = prep.tile([128, NT * K], f32, tag=name)
                return t, t[:].rearrange("p (t k) -> p t k", t=NT)

            py, pyv = t3("py")
            px, pxv = t3("px")
            fy, fyv = t3("fy")
            fx, fxv = t3("fx")
            wy, wyv = t3("wy")
            wx, wxv = t3("wx")
            ta, tav = t3("ta")
            tb, tbv = t3("tb")
            ti = prep.tile([128, NT * K], i32, tag="ti")
            tiv = ti[:].rearrange("p (t k) -> p t k", t=NT)

            V = nc.vector
            # py = dy + (ho - 1 + ky); floor & frac
            V.tensor_tensor(out=pyv, in0=dyv, in1=hov, op=Alu.add)
            V.tensor_copy(out=tiv, in_=pyv)
            V.tensor_copy(out=tav, in_=tiv)
            V.tensor_tensor(out=tbv, in0=tav, in1=pyv, op=Alu.is_gt)
            V.tensor_tensor(out=fyv, in0=tav, in1=tbv, op=Alu.subtract)
            V.tensor_tensor(out=wyv, in0=pyv, in1=fyv, op=Alu.subtract)
            # px = dx + (wo - 1 + kx)
            V.tensor_tensor(out=pxv, in0=dxv, in1=wov, op=Alu.add)
            V.tensor_copy(out=tiv, in_=pxv)
            V.tensor_copy(out=tav, in_=tiv)
            V.tensor_tensor(out=tbv, in0=tav, in1=pxv, op=Alu.is_gt)
            V.tensor_tensor(out=fxv, in0=tav, in1=tbv, op=Alu.subtract)
            V.tensor_tensor(out=wxv, in0=pxv, in1=fxv, op=Alu.subtract)

            vm0, vm0v = t3("vm0")
            vm1, vm1v = t3("vm1")
            vc0, vc0v = t3("vc0")
            vc1, vc1v = t3("vc1")
            cA, cAv = t3("cA")
            cB, cBv = t3("cB")
            # row validity (* mask) and column validity
            V.tensor_scalar(out=tav, in0=fyv, scalar1=0.0, scalar2=None, op0=Alu.is_ge)
            V.tensor_scalar(out=tbv, in0=fyv, scalar1=95.0, scalar2=None, op0=Alu.is_le)
            V.tensor_tensor(out=vm0v, in0=tav, in1=tbv, op=Alu.mult)
            V.tensor_tensor(out=vm0v, in0=vm0v, in1=mv, op=Alu.mult)
            V.tensor_scalar(out=tav, in0=fyv, scalar1=-1.0, scalar2=None, op0=Alu.is_ge)
            V.tensor_scalar(out=tbv, in0=fyv, scalar1=94.0, scalar2=None, op0=Alu.is_le)
            V.tensor_tensor(out=vm1v, in0=tav, in1=tbv, op=Alu.mult)
            V.tensor_tensor(out=vm1v, in0=vm1v, in1=mv, op=Alu.mult)
            V.tensor_scalar(out=tav, in0=fxv, scalar1=0.0, scalar2=None, op0=Alu.is_ge)
            V.tensor_scalar(out=tbv, in0=fxv, scalar1=95.0, scalar2=None, op0=Alu.is_le)
            V.tensor_tensor(out=vc0v, in0=tav, in1=tbv, op=Alu.mult)
            V.tensor_scalar(out=tav, in0=fxv, scalar1=-1.0, scalar2=None, op0=Alu.is_ge)
            V.tensor_scalar(out=tbv, in0=fxv, scalar1=94.0, scalar2=None, op0=Alu.is_le)
            V.tensor_tensor(out=vc1v, in0=tav, in1=tbv, op=Alu.mult)

            # bilinear coefficients: cy0/cy1 (carry mask), cx0/cx1
            nc.scalar.activation(out=tav, in_=wyv, func=Act.Copy, bias=1.0, scale=-1.0)
            V.tensor_tensor(out=cAv, in0=tav, in1=vm0v, op=Alu.mult)   # cy0
            V.tensor_tensor(out=cBv, in0=wyv, in1=vm1v, op=Alu.mult)   # cy1
            nc.scalar.activation(out=tav, in_=wxv, func=Act.Copy, bias=1.0, scale=-1.0)
            V.tensor_tensor(out=vc0v, in0=tav, in1=vc0v, op=Alu.mult)  # cx0
            V.tensor_tensor(out=vc1v, in0=wxv, in1=vc1v, op=Alu.mult)  # cx1

            wq = apool.tile([128, NT * K * 4], f32)
            wq5 = wq[:].rearrange("p (t k l v) -> p t k l v", t=NT, k=K, l=2)
            V.tensor_tensor(out=wq5[:, :, :, 0, 0], in0=cAv, in1=vc0v, op=Alu.mult)
            V.tensor_tensor(out=wq5[:, :, :, 0, 1], in0=cBv, in1=vc0v, op=Alu.mult)
            V.tensor_tensor(out=wq5[:, :, :, 1, 0], in0=cAv, in1=vc1v, op=Alu.mult)
            V.tensor_tensor(out=wq5[:, :, :, 1, 1], in0=cBv, in1=vc1v, op=Alu.mult)

            # quad-gather indices: clamp(96*fy + fx + 97, 0, 9312)
            idxf = prep.tile([128, NT * K], f32)
            idxfv = idxf[:].rearrange("p (t k) -> p t k", t=NT)
            V.scalar_tensor_tensor(out=idxfv, in0=fyv, scalar=96.0, in1=fxv,
                                   op0=Alu.mult, op1=Alu.add)
            V.tensor_scalar(out=idxf[:], in0=idxf[:], scalar1=97.0,
                            scalar2=None, op0=Alu.add)
            V.tensor_scalar(out=idxf[:], in0=idxf[:], scalar1=0.0, scalar2=9312.0,
                            op0=Alu.max, op1=Alu.min)
            idxi = apool.tile([128, NT * K], i32)
            V.tensor_copy(out=idxi[:], in_=idxf[:])
            idxi3 = idxi[:].rearrange("p (t k) -> p t k", t=NT)

            wqv_all = wq[:].rearrange("p (t r) -> p t r", t=NT)

            prep_ctx.close()
            gpool = ctx.enter_context(tc.tile_pool(name="gather", bufs=6))
            wgpool = ctx.enter_context(tc.tile_pool(name="wg", bufs=4))
            spool = ctx.enter_context(tc.tile_pool(name="samp", bufs=4))
            stpool = ctx.enter_context(tc.tile_pool(name="sampT", bufs=3))
            obpool = ctx.enter_context(tc.tile_pool(name="ob", bufs=3))

            # ---- phase B: per-tile gather -> bilinear -> transpose -> matmul ----
            for t in range(NT):
                g = gpool.tile([128, K * 4 * C], f32)
                for k in range(K):
                    nc.gpsimd.indirect_dma_start(
                        out=g[:, k * 4 * C:(k + 1) * 4 * C],
                        out_offset=None,
                        in_=img_pad_ap,
                        in_offset=bass.IndirectOffsetOnAxis(
                            ap=idxi3[:, t, k:k + 1], axis=0),
                    )
                g5 = g[:].rearrange("p (k l v c) -> p k l v c", k=K, l=2, v=2)
                wq_t = wqv_all[:, t, :].rearrange("p (k l v) -> p k l v", k=K, l=2)
                wq_b = wq_t.unsqueeze(4).to_broadcast([128, K, 2, 2, C])
                wg = wgpool.tile([128, K * 4 * C], f32)
                wg5 = wg[:].rearrange("p (k l v c) -> p k l v c", k=K, l=2, v=2)
                V.tensor_tensor(out=wg5, in0=g5, in1=wq_b, op=Alu.mult)

                s01 = spool.tile([128, K * 2 * C], f32, tag="s01")
                s013 = s01[:].rearrange("p (k v c) -> p k v c", k=K, v=2)
                V.tensor_tensor(out=s013, in0=wg5[:, :, 0, :, :],
                                in1=wg5[:, :, 1, :, :], op=Alu.add)
                samp = spool.tile([128, K * C], f32)
                samp3 = samp[:].rearrange("p (k c) -> p k c", k=K)
                V.tensor_tensor(out=samp3, in0=s013[:, :, 0, :],
                                in1=s013[:, :, 1, :], op=Alu.add)

                sampT = stpool.tile([128, 5 * 128], f32)
                for ci, off in enumerate(CHUNK_OFFS):
                    pt = tb_ps.tile([128, 128], f32, tag="trB")
                    nc.tensor.transpose(out=pt[:], in_=samp[:, off:off + 128],
                                        identity=ident[:])
                    nc.scalar.activation(
                        out=sampT[:, ci * 128:(ci + 1) * 128], in_=pt[:],
                        func=Act.Copy)

                po = opsum.tile([Co, 128], f32)
                w2v = w2_sb[:].rearrange("p (f c) -> p f c", f=5)
                for ci in range(5):
                    nc.tensor.matmul(
                        out=po[:], lhsT=w2v[:, ci, :],
                        rhs=sampT[:, ci * 128:(ci + 1) * 128],
                        start=(ci == 0), stop=(ci == 4))

                ob = obpool.tile([Co, 128], f32)
                V.tensor_scalar(out=ob[:], in0=po[:], scalar1=bias_sb[:, 0:1],
                                scalar2=None, op0=Alu.add)
                nc.sync.dma_start(out=out_ap[:, t * 128:(t + 1) * 128], in_=ob[:])

    nc.compile()
    nc.m = get_hw_module(nc.m)
    return nc


def _host_prep(input, offset, mask, weight, bias):
    f32 = np.float32
    input = np.ascontiguousarray(input, dtype=f32)
    offset = np.ascontiguousarray(offset, dtype=f32)
    mask = np.ascontiguousarray(mask, dtype=f32)
    weight = np.ascontiguousarray(weight, dtype=f32)
    bias = np.ascontiguousarray(bias, dtype=f32)

    # weight [Co, C, 3, 3] -> W2r[(t*64+c), co], chunked at CHUNK_OFFS with
    # the 448-overlap region zeroed out of chunk 4 (rows 448..511 live in
    # chunk 3).
    wr = weight.reshape(Co, C, K)                     # [co, c, t]
    W2r = np.transpose(wr, (2, 1, 0)).reshape(C * K, Co)  # [(t,c), co]
    w2 = np.zeros((5, 128, Co), dtype=f32)
    w2[0] = W2r[0:128]
    w2[1] = W2r[128:256]
    w2[2] = W2r[256:384]
    w2[3] = W2r[384:512]
    w2[4, 64:128] = W2r[512:576]
    w2 = w2.reshape(5 * 128, Co)

    biasv = bias.reshape(Co, 1)
    kyv = (np.arange(K, dtype=f32) // 3)
    kxv = (np.arange(K, dtype=f32) % 3)

    pix = np.arange(NPIX).reshape(NT, 128)
    in_maps = []
    for core in range(N_CORES):
        b, h = core // 2, core % 2
        ho0 = h * HHALF
        ho = ho0 + pix // W
        wo = pix % W
        base_y = (ho - 1)[:, :, None] + kyv[None, None, :]   # [NT, 128, K]
        base_x = (wo - 1)[:, :, None] + kxv[None, None, :]
        byx = np.stack([base_y, base_x], axis=-1)            # [NT, 128, K, 2]
        byx = np.ascontiguousarray(
            byx.transpose(1, 0, 2, 3).reshape(128, NT * K * 2), dtype=f32)
        offmask = np.concatenate(
            [offset[b, :, ho0:ho0 + HHALF, :].reshape(18, NPIX),
             mask[b, :, ho0:ho0 + HHALF, :].reshape(K, NPIX)], axis=0)
        in_maps.append({
            "img": input[b].reshape(C, HW),
            "offmask": np.ascontiguousarray(offmask),
            "byx": byx,
            "w2": w2,
            "biasv": biasv,
        })
    return in_maps


def kernel(input, offset, mask, weight, bias):
    from concourse.bass_utils import run_bass_kernel_spmd

    if "nc" not in _CACHE:
        _CACHE["nc"] = _build_module()
    nc = _CACHE["nc"]

    in_maps = _host_prep(input, offset, mask, weight, bias)
    res = run_bass_kernel_spmd(nc, in_maps, core_ids=list(range(N_CORES)))

    out = np.empty((B, Co, H, W), dtype=np.float32)
    for core in range(N_CORES):
        b, h = core // 2, core % 2
        ho0 = h * HHALF
        out[b, :, ho0:ho0 + HHALF, :] = \
            res.results[core]["out"].reshape(Co, HHALF, W)
    return out



# revision 8
# speedup vs baseline: 1.3156x; 1.3156x over previous
"""DCNv2 (modulated deformable conv) forward on 8 Trainium2 NeuronCores.

Problem: input [4,64,96,96], offset [4,18,96,96], mask [4,9,96,96],
weight [64,64,3,3], bias [64] -> out [4,64,96,96]. STRIDE=1, PAD=1, DIL=1,
deformable groups G=1.

Sharding: data-parallel over (batch, H-half): core = b*2 + h handles output
rows [48h, 48h+48) of batch b. Each core receives the full image of its
batch (sampling positions are unbounded so no halo margin is safe), plus
its slice of offset/mask; weight/bias replicated.

Per-core device algorithm (all compute on device):
  1. Image [64, 9216] -> pixel-major quad-packed imgPad [9314, 128] bf16 in
     DRAM: imgPad[r] = [pixel(r-97) | pixel(r-1)]; rows 0/9313 zero pads.
     One 256-elem read at row r = 96*fy+fx+97 (stride 128) returns all four
     bilinear neighbors [v00, v10, v01, v11] of a sample.
  2. Transpose offset/mask per 128-pixel tile, compute sampling positions,
     floor/frac, validity masks, bilinear*mask weights wq, and clamped
     quad indices idxf (f32).
  3. idxf [128px, (t,k)] is rewrapped into the SWDGE dma_gather index
     layout (index j at partition j%16, free j//16, replicated x8) via 8
     identity-slice matmuls (PSUM [16, NT*K]) + strided f32->i16 copies +
     7 partition-replication DMAs.
  4. Gathers run per (tap, 8-tile group): one InstDMAGatherAnt with
     num_idxs = 1024 (<= the 1024-descriptor SWDGE ring limit; 41 gathers
     total instead of 324 indirect DMAs at ~1.1us fixed SWDGE cost each).
     dst [128px, 8t, 256] bf16. Vector engine applies bilinear weights and
     reduces the 4 neighbors -> samp [128px, t, 9*64] bf16; PE transposes
     per tile (chunks 0,128,256,384,448 dedup'd in the weight layout) and
     contracts with the rearranged bf16 weight into out [64co, 128px] f32;
     bias added from PSUM; HWDGE stores.

Measured on 8 axon trn2 cores (f32 variant): rel err 2.9e-6; bf16 variant
trades ~1e-3 rel err for half the gather HBM traffic and 2x DVE/PE rates.
"""

import os
import sys
import types
import dataclasses
import numpy as np

for _p in ("/opt/trn_rl_repo",):
    if _p not in sys.path and os.path.isdir(_p):
        sys.path.append(_p)

try:
    import antenv.axon_hooks  # noqa: F401
except ImportError:
    _hookmod = types.ModuleType("antenv.axon_hooks")
    _hookmod._hook = None
    _hookmod.set_axon_ntff_profile_hook = lambda h: setattr(_hookmod, "_hook", h)
    _hookmod.get_axon_ntff_profile_hook = lambda: _hookmod._hook
    sys.modules["antenv.axon_hooks"] = _hookmod

B, C, H, W = 4, 64, 96, 96
K = 9
Co = 64
HW = H * W                  # 9216
N_CORES = 8
HHALF = 48
NPIX = HHALF * W            # 4608 output pixels per core
NT = NPIX // 128            # 36 tiles
CHUNK_OFFS = (0, 128, 256, 384, 448)  # samp free-dim transpose chunks
GROUPS = ((0, 8), (8, 8), (16, 8), (24, 8), (32, 4))  # (tile0, ntiles)
DT16 = True                 # bf16 gather/weighting path

_CACHE = {}


def _build_module():
    from contextlib import ExitStack

    import concourse.bass as bass
    import concourse.tile as tile
    from concourse import bacc, mybir
    from concourse.bass_interp import get_hw_module
    from concourse.masks import make_identity

    f32 = mybir.dt.float32
    i16 = mybir.dt.int16
    dtv = mybir.dt.bfloat16 if DT16 else f32
    Alu = mybir.AluOpType
    Act = mybir.ActivationFunctionType

    nc = bacc.Bacc("TRN2", target_bir_lowering=False, debug=False,
                   enable_asserts=False, num_devices=N_CORES)

    img_ap = nc.dram_tensor("img", [C, HW], f32, kind="ExternalInput").ap()
    offmask_ap = nc.dram_tensor("offmask", [27, NPIX], f32, kind="ExternalInput").ap()
    byx_ap = nc.dram_tensor("byx", [128, NT * K * 2], f32, kind="ExternalInput").ap()
    w2_ap = nc.dram_tensor("w2", [5 * 128, Co], dtv, kind="ExternalInput").ap()
    bias_ap = nc.dram_tensor("biasv", [Co, 1], f32, kind="ExternalInput").ap()
    out_ap = nc.dram_tensor("out", [Co, NPIX], f32, kind="ExternalOutput").ap()
    # imgPad[r, 0:64] = pixel(r-97), imgPad[r, 64:128] = pixel(r-1); zero pads.
    # A 256-elem read at row r=q00+97 (row stride 128) yields [v00,v10,v01,v11].
    img_pad_t = nc.dram_tensor("imgpad", [HW + 98, 2 * C], dtv)
    img_pad_ap = img_pad_t.ap()
    # overlapping gather view: row r = 256 contiguous elems from offset 128*r
    img_pad_ov = dataclasses.replace(
        img_pad_ap, ap=[[128, HW + 97], [1, 256]])

    with tile.TileContext(nc) as tc:
        with ExitStack() as ctx:
            cpool = ctx.enter_context(tc.tile_pool(name="consts", bufs=1))
            apool = ctx.enter_context(tc.tile_pool(name="phase_a", bufs=1))
            prep_ctx = ExitStack()
            prep = prep_ctx.enter_context(tc.tile_pool(name="prep", bufs=1))
            tp_ps = ctx.enter_context(tc.tile_pool(name="tr_ps", bufs=3, space="PSUM"))
            sel_ps = ctx.enter_context(tc.tile_pool(name="sel_ps", bufs=1, space="PSUM"))
            tb_ps = ctx.enter_context(tc.tile_pool(name="trB_ps", bufs=2, space="PSUM"))
            opsum = ctx.enter_context(tc.tile_pool(name="opsum", bufs=2, space="PSUM"))

            # ---- constants ----
            ident = cpool.tile([128, 128], f32)
            make_identity(nc, ident[:])
            identB = ident
            if DT16:
                identB = cpool.tile([128, 128], dtv)
                make_identity(nc, identB[:])
            # offset/mask load first: it gates the A2->A3->gather chain
            om = prep.tile([27, NPIX], f32)
            nc.sync.dma_start(out=om[:], in_=offmask_ap)
            byx_sb = cpool.tile([128, NT * K * 2], f32)
            nc.sync.dma_start(out=byx_sb[:], in_=byx_ap)

            zt = cpool.tile([128, C], dtv)
            nc.vector.memset(zt[:], 0.0)
            nc.sync.dma_start(out=img_pad_ap[0:97, 0:C], in_=zt[0:97, :])
            nc.sync.dma_start(out=img_pad_ap[0:1, C:2 * C], in_=zt[0:1, :])
            nc.sync.dma_start(out=img_pad_ap[HW + 97:HW + 98, 0:C], in_=zt[0:1, :])
            nc.sync.dma_start(out=img_pad_ap[HW + 1:HW + 98, C:2 * C], in_=zt[0:97, :])

            w2_sb = cpool.tile([128, 5 * Co], dtv)
            nc.sync.dma_start(
                out=w2_sb[:].rearrange("p (f c) -> p f c", f=5),
                in_=w2_ap.rearrange("(f p) c -> p f c", p=128),
            )
            bias_sb = cpool.tile([Co, 1], f32)
            nc.sync.dma_start(out=bias_sb[:], in_=bias_ap)

            # ---- phase A2: offset/mask tile transposes ----
            omT = prep.tile([128, NT * 27], f32)
            for t in range(NT):
                pt = tp_ps.tile([128, 27], f32, tag="tr")
                nc.tensor.transpose(
                    out=pt[:], in_=om[:, t * 128:(t + 1) * 128],
                    identity=ident[:27, :27])
                nc.scalar.activation(
                    out=omT[:, t * 27:(t + 1) * 27], in_=pt[:], func=Act.Copy)

            # ---- phase A1: image -> pixel-major quad-packed imgPad, written
            # in chunks so DMA overlaps the PE transposes ----
            s_img = prep.tile([C, HW], f32)
            for g0 in range(0, 72, 12):
                nc.sync.dma_start(out=s_img[:, g0 * 128:(g0 + 12) * 128],
                                  in_=img_ap[:, g0 * 128:(g0 + 12) * 128])
            s_imgT = prep.tile([128, 72 * C], dtv)
            CH = 12
            for g0 in range(0, 72, CH):
                for j in range(g0, g0 + CH):
                    pt = tp_ps.tile([128, C], f32, tag="tr")
                    nc.tensor.transpose(
                        out=pt[:], in_=s_img[:, j * 128:(j + 1) * 128],
                        identity=ident[:C, :C])
                    if j % 2 == 0:
                        nc.scalar.activation(
                            out=s_imgT[:, j * C:(j + 1) * C], in_=pt[:],
                            func=Act.Copy)
                    else:
                        nc.vector.tensor_copy(
                            out=s_imgT[:, j * C:(j + 1) * C], in_=pt[:])
                r0 = g0 * 128
                r1 = (g0 + CH) * 128
                seg = s_imgT[:, g0 * C:(g0 + CH) * C].rearrange(
                    "p (j c) -> p j c", j=CH)
                nc.sync.dma_start(
                    out=img_pad_ap[97 + r0:97 + r1, 0:C].rearrange(
                        "(j p) c -> p j c", p=128),
                    in_=seg)
                nc.sync.dma_start(
                    out=img_pad_ap[1 + r0:1 + r1, C:2 * C].rearrange(
                        "(j p) c -> p j c", p=128),
                    in_=seg)

            # ---- phase A3: index & weight math (batched over all tiles) ----
            omT3 = omT[:].rearrange("p (t c) -> p t c", t=NT)
            dyv = omT3[:, :, 0:18:2]          # [128, NT, 9]
            dxv = omT3[:, :, 1:18:2]
            mv = omT3[:, :, 18:27]
            byx4 = byx_sb[:].rearrange("p (t k s) -> p t k s", t=NT, k=K)
            hov = byx4[:, :, :, 0]          # ho - 1 + ky  [128, NT, K]
            wov = byx4[:, :, :, 1]          # wo - 1 + kx

            def t3(name):
                t = prep.tile([128, NT * K], f32, tag=name)
                return t, t[:].rearrange("p (t k) -> p t k", t=NT)

            py, pyv = t3("py")
            px, pxv = t3("px")
            fy, fyv = t3("fy")
            fx, fxv = t3("fx")
            wy, wyv = t3("wy")
            wx, wxv = t3("wx")
            ta, tav = t3("ta")
            tb, tbv = t3("tb")
            ti = prep.tile([128, NT * K], mybir.dt.int32, tag="ti")
            tiv = ti[:].rearrange("p (t k) -> p t k", t=NT)

            V = nc.vector
            # py = dy + (ho - 1 + ky); floor & frac
            V.tensor_tensor(out=pyv, in0=dyv, in1=hov, op=Alu.add)
            V.tensor_copy(out=tiv, in_=pyv)
            V.tensor_copy(out=tav, in_=tiv)
            V.tensor_tensor(out=tbv, in0=tav, in1=pyv, op=Alu.is_gt)
            V.tensor_tensor(out=fyv, in0=tav, in1=tbv, op=Alu.subtract)
            V.tensor_tensor(out=wyv, in0=pyv, in1=fyv, op=Alu.subtract)
            # px = dx + (wo - 1 + kx)
            V.tensor_tensor(out=pxv, in0=dxv, in1=wov, op=Alu.add)
            V.tensor_copy(out=tiv, in_=pxv)
            V.tensor_copy(out=tav, in_=tiv)
            V.tensor_tensor(out=tbv, in0=tav, in1=pxv, op=Alu.is_gt)
            V.tensor_tensor(out=fxv, in0=tav, in1=tbv, op=Alu.subtract)
            V.tensor_tensor(out=wxv, in0=pxv, in1=fxv, op=Alu.subtract)

            vm0, vm0v = t3("vm0")
            vm1, vm1v = t3("vm1")
            vc0, vc0v = t3("vc0")
            vc1, vc1v = t3("vc1")
            cA, cAv = t3("cA")
            cB, cBv = t3("cB")
            # row validity (* mask) and column validity
            V.tensor_scalar(out=tav, in0=fyv, scalar1=0.0, scalar2=None, op0=Alu.is_ge)
            V.tensor_scalar(out=tbv, in0=fyv, scalar1=95.0, scalar2=None, op0=Alu.is_le)
            V.tensor_tensor(out=vm0v, in0=tav, in1=tbv, op=Alu.mult)
            V.tensor_tensor(out=vm0v, in0=vm0v, in1=mv, op=Alu.mult)
            V.tensor_scalar(out=tav, in0=fyv, scalar1=-1.0, scalar2=None, op0=Alu.is_ge)
            V.tensor_scalar(out=tbv, in0=fyv, scalar1=94.0, scalar2=None, op0=Alu.is_le)
            V.tensor_tensor(out=vm1v, in0=tav, in1=tbv, op=Alu.mult)
            V.tensor_tensor(out=vm1v, in0=vm1v, in1=mv, op=Alu.mult)
            V.tensor_scalar(out=tav, in0=fxv, scalar1=0.0, scalar2=None, op0=Alu.is_ge)
            V.tensor_scalar(out=tbv, in0=fxv, scalar1=95.0, scalar2=None, op0=Alu.is_le)
            V.tensor_tensor(out=vc0v, in0=tav, in1=tbv, op=Alu.mult)
            V.tensor_scalar(out=tav, in0=fxv, scalar1=-1.0, scalar2=None, op0=Alu.is_ge)
            V.tensor_scalar(out=tbv, in0=fxv, scalar1=94.0, scalar2=None, op0=Alu.is_le)
            V.tensor_tensor(out=vc1v, in0=tav, in1=tbv, op=Alu.mult)

            # bilinear coefficients: cy0/cy1 (carry mask), cx0/cx1
            nc.scalar.activation(out=tav, in_=wyv, func=Act.Copy, bias=1.0, scale=-1.0)
            V.tensor_tensor(out=cAv, in0=tav, in1=vm0v, op=Alu.mult)   # cy0
            V.tensor_tensor(out=cBv, in0=wyv, in1=vm1v, op=Alu.mult)   # cy1
            nc.scalar.activation(out=tav, in_=wxv, func=Act.Copy, bias=1.0, scale=-1.0)
            V.tensor_tensor(out=vc0v, in0=tav, in1=vc0v, op=Alu.mult)  # cx0
            V.tensor_tensor(out=vc1v, in0=wxv, in1=vc1v, op=Alu.mult)  # cx1

            wq = apool.tile([128, NT * K * 4], dtv)
            wq5 = wq[:].rearrange("p (t k l v) -> p t k l v", t=NT, k=K, l=2)
            V.tensor_tensor(out=wq5[:, :, :, 0, 0], in0=cAv, in1=vc0v, op=Alu.mult)
            V.tensor_tensor(out=wq5[:, :, :, 0, 1], in0=cBv, in1=vc0v, op=Alu.mult)
            V.tensor_tensor(out=wq5[:, :, :, 1, 0], in0=cAv, in1=vc1v, op=Alu.mult)
            V.tensor_tensor(out=wq5[:, :, :, 1, 1], in0=cBv, in1=vc1v, op=Alu.mult)

            # quad-gather indices: clamp(96*fy + fx + 97, 0, 9312), f32
            idxf = prep.tile([128, NT * K], f32)
            idxfv = idxf[:].rearrange("p (t k) -> p t k", t=NT)
            V.scalar_tensor_tensor(out=idxfv, in0=fyv, scalar=96.0, in1=fxv,
                                   op0=Alu.mult, op1=Alu.add)
            V.tensor_scalar(out=idxf[:], in0=idxf[:], scalar1=97.0,
                            scalar2=None, op0=Alu.add)
            V.tensor_scalar(out=idxf[:], in0=idxf[:], scalar1=0.0, scalar2=9312.0,
                            op0=Alu.max, op1=Alu.min)

            # ---- wrapped SWDGE index layout: idx16[p%16, k*288 + t*8 + p//16]
            # (replicated x8 across 16-partition groups) ----
            idx16 = apool.tile([128, K * NT * 8], i16)
            idx16v = idx16[:].rearrange("p (k t q) -> p k t q", k=K, t=NT)
            for p1 in range(8):
                ps = sel_ps.tile([16, NT * K], f32, tag="sel")
                nc.tensor.matmul(
                    out=ps[:], lhsT=ident[:, 16 * p1:16 * (p1 + 1)],
                    rhs=idxf[:], start=True, stop=True)
                psv = ps[:].rearrange("p (t k) -> p t k", t=NT)
                V.tensor_copy(
                    out=idx16v[0:16, :, :, p1].rearrange("p k t -> p t k"),
                    in_=psv)
            for g_ in range(1, 8):
                nc.sync.dma_start(out=idx16[16 * g_:16 * (g_ + 1), :],
                                  in_=idx16[0:16, :])

            prep_ctx.close()
            gpool = ctx.enter_context(tc.tile_pool(name="gather", bufs=2))
            spool = ctx.enter_context(tc.tile_pool(name="samp", bufs=2))
            stpool = ctx.enter_context(tc.tile_pool(name="sampT", bufs=3))
            obpool = ctx.enter_context(tc.tile_pool(name="ob", bufs=3))

            # ---- phase B: per-(group, tap) gather -> bilinear; per-tile
            # transpose -> matmul ----
            wq5v = wq[:].rearrange("p (t k l v) -> p t k l v", t=NT, k=K, l=2)
            w2v = w2_sb[:].rearrange("p (f c) -> p f c", f=5)
            for t0, gt in GROUPS:
                gk = gpool.tile([128, gt * K * 256], dtv, tag="gk")
                gk4 = gk[:].rearrange("p (k t e) -> p k t e", k=K, t=gt)
                for k in range(K):
                    nc.gpsimd.dma_gather(
                        out_ap=gk4[:, k, :, :],
                        in_ap=img_pad_ov,
                        idxs_ap=idx16v[:, k, t0:t0 + gt, :].rearrange(
                            "p t q -> p (t q)"),
                        num_idxs=gt * 128,
                        num_idxs_reg=gt * 128,
                        elem_size=256,
                        elem_step=128,
                    )
                samp = spool.tile([128, gt * K * C], dtv, tag="samp")
                samp4 = samp[:].rearrange("p (t k c) -> p t k c", t=gt, k=K)
                for k in range(K):
                    g5 = gk4[:, k, :, :].rearrange(
                        "p t (l v c) -> p t l v c", l=2, v=2)
                    wq_b = wq5v[:, t0:t0 + gt, k, :, :].unsqueeze(4).to_broadcast(
                        [128, gt, 2, 2, C])
                    wg = spool.tile([128, gt * 4 * C], dtv, tag="wg")
                    wg5 = wg[:].rearrange("p (t l v c) -> p t l v c",
                                          t=gt, l=2, v=2)
                    V.tensor_tensor(out=wg5, in0=g5, in1=wq_b, op=Alu.mult)
                    s01 = spool.tile([128, gt * 2 * C], dtv, tag="s01")
                    s013 = s01[:].rearrange("p (t v c) -> p t v c", t=gt, v=2)
                    V.tensor_tensor(out=s013, in0=wg5[:, :, 0, :, :],
                                    in1=wg5[:, :, 1, :, :], op=Alu.add)
                    V.tensor_tensor(out=samp4[:, :, k, :], in0=s013[:, :, 0, :],
                                    in1=s013[:, :, 1, :], op=Alu.add)

                samp_fl = samp[:].rearrange("p (t x) -> p t x", t=gt)
                for tl in range(gt):
                    sampT = stpool.tile([128, 5 * 128], dtv)
                    for ci, off in enumerate(CHUNK_OFFS):
                        pt = tb_ps.tile([128, 128], dtv, tag="trB")
                        nc.tensor.transpose(
                            out=pt[:], in_=samp_fl[:, tl, off:off + 128],
                            identity=identB[:])
                        nc.scalar.activation(
                            out=sampT[:, ci * 128:(ci + 1) * 128], in_=pt[:],
                            func=Act.Copy)

                    po = opsum.tile([Co, 128], f32)
                    for ci in range(5):
                        nc.tensor.matmul(
                            out=po[:], lhsT=w2v[:, ci, :],
                            rhs=sampT[:, ci * 128:(ci + 1) * 128],
                            start=(ci == 0), stop=(ci == 4))

                    t = t0 + tl
                    ob = obpool.tile([Co, 128], f32)
                    V.tensor_scalar(out=ob[:], in0=po[:], scalar1=bias_sb[:, 0:1],
                                    scalar2=None, op0=Alu.add)
                    nc.sync.dma_start(out=out_ap[:, t * 128:(t + 1) * 128],
                                      in_=ob[:])

    nc.compile()
    nc.m = get_hw_module(nc.m)
    return nc


def _host_prep(input, offset, mask, weight, bias):
    import ml_dtypes
    f32 = np.float32
    input = np.ascontiguousarray(input, dtype=f32)
    offset = np.ascontiguousarray(offset, dtype=f32)
    mask = np.ascontiguousarray(mask, dtype=f32)
    weight = np.ascontiguousarray(weight, dtype=f32)
    bias = np.ascontiguousarray(bias, dtype=f32)

    # weight [Co, C, 3, 3] -> W2r[(t*64+c), co], chunked at CHUNK_OFFS with
    # the 448-overlap region zeroed out of chunk 4 (rows 448..511 live in
    # chunk 3).
    wr = weight.reshape(Co, C, K)                     # [co, c, t]
    W2r = np.transpose(wr, (2, 1, 0)).reshape(C * K, Co)  # [(t,c), co]
    w2 = np.zeros((5, 128, Co), dtype=f32)
    w2[0] = W2r[0:128]
    w2[1] = W2r[128:256]
    w2[2] = W2r[256:384]
    w2[3] = W2r[384:512]
    w2[4, 64:128] = W2r[512:576]
    w2 = w2.reshape(5 * 128, Co)
    if DT16:
        w2 = w2.astype(ml_dtypes.bfloat16)

    biasv = bias.reshape(Co, 1)
    kyv = (np.arange(K, dtype=f32) // 3)
    kxv = (np.arange(K, dtype=f32) % 3)

    pix = np.arange(NPIX).reshape(NT, 128)
    in_maps = []
    for core in range(N_CORES):
        b, h = core // 2, core % 2
        ho0 = h * HHALF
        ho = ho0 + pix // W
        wo = pix % W
        base_y = (ho - 1)[:, :, None] + kyv[None, None, :]   # [NT, 128, K]
        base_x = (wo - 1)[:, :, None] + kxv[None, None, :]
        byx = np.stack([base_y, base_x], axis=-1)            # [NT, 128, K, 2]
        byx = np.ascontiguousarray(
            byx.transpose(1, 0, 2, 3).reshape(128, NT * K * 2), dtype=f32)
        offmask = np.concatenate(
            [offset[b, :, ho0:ho0 + HHALF, :].reshape(18, NPIX),
             mask[b, :, ho0:ho0 + HHALF, :].reshape(K, NPIX)], axis=0)
        in_maps.append({
            "img": input[b].reshape(C, HW),
            "offmask": np.ascontiguousarray(offmask),
            "byx": byx,
            "w2": w2,
            "biasv": biasv,
        })
    return in_maps


def kernel(input, offset, mask, weight, bias):
    from concourse.bass_utils import run_bass_kernel_spmd

    if "nc" not in _CACHE:
        _CACHE["nc"] = _build_module()
    nc = _CACHE["nc"]

    in_maps = _host_prep(input, offset, mask, weight, bias)
    res = run_bass_kernel_spmd(nc, in_maps, core_ids=list(range(N_CORES)))

    out = np.empty((B, Co, H, W), dtype=np.float32)
    for core in range(N_CORES):
        b, h = core // 2, core % 2
        ho0 = h * HHALF
        out[b, :, ho0:ho0 + HHALF, :] = \
            res.results[core]["out"].reshape(Co, HHALF, W)
    return out


# revision 10
# speedup vs baseline: 2.1195x; 1.6110x over previous
"""DCNv2 (modulated deformable conv) forward on 8 Trainium2 NeuronCores.

Problem: input [4,64,96,96], offset [4,18,96,96], mask [4,9,96,96],
weight [64,64,3,3], bias [64] -> out [4,64,96,96]. STRIDE=1, PAD=1, DIL=1,
deformable groups G=1.

Sharding: data-parallel over (batch, H-half): core = b*2 + h handles output
rows [48h, 48h+48) of batch b. Each core receives the full image of its
batch (sampling positions are unbounded so no halo margin is safe), plus
its slice of offset/mask; weight/bias replicated.

Per-core device algorithm (all compute on device):
  1. Image [64, 9216] -> pixel-major quad-packed imgPad [9314, 128] bf16 in
     DRAM: imgPad[r] = [pixel(r-97) | pixel(r-1)]; rows 0/9313 zero pads.
     One 256-elem read at row r = 96*fy+fx+97 (stride 128) returns all four
     bilinear neighbors [v00, v10, v01, v11] of a sample.
  2. Transpose offset/mask per 128-pixel tile, compute sampling positions,
     floor/frac, validity masks, bilinear*mask weights wq, and clamped
     quad indices idxf (f32).
  3. idxf [128px, (t,k)] is rewrapped into the SWDGE dma_gather index
     layout (index j at partition j%16, free j//16, replicated x8) via 8
     identity-slice matmuls (PSUM [16, NT*K]) + strided f32->i16 copies +
     7 partition-replication DMAs.
  4. Gathers run per (tap, 8-tile group): one InstDMAGatherAnt with
     num_idxs = 1024 (<= the 1024-descriptor SWDGE ring limit; 41 gathers
     total instead of 324 indirect DMAs at ~1.1us fixed SWDGE cost each).
     dst [128px, 8t, 256] bf16. Vector engine applies bilinear weights and
     reduces the 4 neighbors -> samp [128px, t, 9*64] bf16; PE transposes
     per tile (chunks 0,128,256,384,448 dedup'd in the weight layout) and
     contracts with the rearranged bf16 weight into out [64co, 128px] f32;
     bias added from PSUM; HWDGE stores.

Measured on 8 axon trn2 cores (f32 variant): rel err 2.9e-6; bf16 variant
trades ~1e-3 rel err for half the gather HBM traffic and 2x DVE/PE rates.
"""

import os
import sys
import types
import dataclasses
import numpy as np

for _p in ("/opt/trn_rl_repo",):
    if _p not in sys.path and os.path.isdir(_p):
        sys.path.append(_p)

try:
    import antenv.axon_hooks  # noqa: F401
except ImportError:
    _hookmod = types.ModuleType("antenv.axon_hooks")
    _hookmod._hook = None
    _hookmod.set_axon_ntff_profile_hook = lambda h: setattr(_hookmod, "_hook", h)
    _hookmod.get_axon_ntff_profile_hook = lambda: _hookmod._hook
    sys.modules["antenv.axon_hooks"] = _hookmod

B, C, H, W = 4, 64, 96, 96
K = 9
Co = 64
HW = H * W                  # 9216
N_CORES = 8
HHALF = 48
NPIX = HHALF * W            # 4608 output pixels per core
NT = NPIX // 128            # 36 tiles
CHUNK_OFFS = (0, 128, 256, 384, 448)  # samp free-dim transpose chunks
GROUPS = ((0, 8), (8, 8), (16, 8), (24, 8), (32, 4))  # (tile0, ntiles)
DT16 = True                 # bf16 gather/weighting path

_CACHE = {}


def _build_module():
    from contextlib import ExitStack

    import concourse.bass as bass
    import concourse.tile as tile
    from concourse import bacc, mybir
    from concourse.bass_interp import get_hw_module
    from concourse.masks import make_identity

    f32 = mybir.dt.float32
    i16 = mybir.dt.int16
    dtv = mybir.dt.bfloat16 if DT16 else f32
    Alu = mybir.AluOpType
    Act = mybir.ActivationFunctionType

    nc = bacc.Bacc("TRN2", target_bir_lowering=False, debug=False,
                   enable_asserts=False, num_devices=N_CORES,
                   num_swdge_queues=4)

    img_ap = nc.dram_tensor("img", [C, HW], f32, kind="ExternalInput").ap()
    offmask_ap = nc.dram_tensor("offmask", [27, NPIX], f32, kind="ExternalInput").ap()
    byx_ap = nc.dram_tensor("byx", [128, NT * K * 2], f32, kind="ExternalInput").ap()
    w2_ap = nc.dram_tensor("w2", [5 * 128, Co], dtv, kind="ExternalInput").ap()
    bias_ap = nc.dram_tensor("biasv", [Co, 1], f32, kind="ExternalInput").ap()
    out_ap = nc.dram_tensor("out", [Co, NPIX], f32, kind="ExternalOutput").ap()
    # imgPad[r, 0:64] = pixel(r-97), imgPad[r, 64:128] = pixel(r-1); zero pads.
    # A 256-elem read at row r=q00+97 (row stride 128) yields [v00,v10,v01,v11].
    img_pad_t = nc.dram_tensor("imgpad", [HW + 98, 2 * C], dtv)
    img_pad_ap = img_pad_t.ap()
    # overlapping gather view: row r = 256 contiguous elems from offset 128*r
    img_pad_ov = dataclasses.replace(
        img_pad_ap, ap=[[128, HW + 97], [1, 256]])

    with tile.TileContext(nc) as tc:
        with ExitStack() as ctx:
            cpool = ctx.enter_context(tc.tile_pool(name="consts", bufs=1))
            apool = ctx.enter_context(tc.tile_pool(name="phase_a", bufs=1))
            prep_ctx = ExitStack()
            prep = prep_ctx.enter_context(tc.tile_pool(name="prep", bufs=1))
            tp_ps = ctx.enter_context(tc.tile_pool(name="tr_ps", bufs=3, space="PSUM"))
            sel_ps = ctx.enter_context(tc.tile_pool(name="sel_ps", bufs=1, space="PSUM"))
            tb_ps = ctx.enter_context(tc.tile_pool(name="trB_ps", bufs=2, space="PSUM"))
            opsum = ctx.enter_context(tc.tile_pool(name="opsum", bufs=2, space="PSUM"))

            # ---- constants ----
            ident = cpool.tile([128, 128], f32)
            make_identity(nc, ident[:])
            identB = ident
            if DT16:
                identB = cpool.tile([128, 128], dtv)
                make_identity(nc, identB[:])
            # offset/mask load first: it gates the A2->A3->gather chain
            om = prep.tile([27, NPIX], f32)
            nc.sync.dma_start(out=om[:], in_=offmask_ap)
            byx_sb = cpool.tile([128, NT * K * 2], f32)
            nc.sync.dma_start(out=byx_sb[:], in_=byx_ap)

            zt = cpool.tile([128, C], dtv)
            nc.vector.memset(zt[:], 0.0)
            nc.sync.dma_start(out=img_pad_ap[0:97, 0:C], in_=zt[0:97, :])
            nc.sync.dma_start(out=img_pad_ap[0:1, C:2 * C], in_=zt[0:1, :])
            nc.sync.dma_start(out=img_pad_ap[HW + 97:HW + 98, 0:C], in_=zt[0:1, :])
            nc.sync.dma_start(out=img_pad_ap[HW + 1:HW + 98, C:2 * C], in_=zt[0:97, :])

            w2_sb = cpool.tile([128, 5 * Co], dtv)
            nc.sync.dma_start(
                out=w2_sb[:].rearrange("p (f c) -> p f c", f=5),
                in_=w2_ap.rearrange("(f p) c -> p f c", p=128),
            )
            bias_sb = cpool.tile([Co, 1], f32)
            nc.sync.dma_start(out=bias_sb[:], in_=bias_ap)

            # ---- phase A2: offset/mask tile transposes ----
            omT = prep.tile([128, NT * 27], f32)
            for t in range(NT):
                pt = tp_ps.tile([128, 27], f32, tag="tr")
                nc.tensor.transpose(
                    out=pt[:], in_=om[:, t * 128:(t + 1) * 128],
                    identity=ident[:27, :27])
                nc.scalar.activation(
                    out=omT[:, t * 27:(t + 1) * 27], in_=pt[:], func=Act.Copy)

            # ---- phase A1: image -> pixel-major quad-packed imgPad, written
            # in chunks so DMA overlaps the PE transposes ----
            s_img = prep.tile([C, HW], f32)
            for g0 in range(0, 72, 12):
                nc.sync.dma_start(out=s_img[:, g0 * 128:(g0 + 12) * 128],
                                  in_=img_ap[:, g0 * 128:(g0 + 12) * 128])
            s_imgT = prep.tile([128, 72 * C], dtv)
            CH = 12
            for g0 in range(0, 72, CH):
                for j in range(g0, g0 + CH):
                    pt = tp_ps.tile([128, C], f32, tag="tr")
                    nc.tensor.transpose(
                        out=pt[:], in_=s_img[:, j * 128:(j + 1) * 128],
                        identity=ident[:C, :C])
                    if j % 2 == 0:
                        nc.scalar.activation(
                            out=s_imgT[:, j * C:(j + 1) * C], in_=pt[:],
                            func=Act.Copy)
                    else:
                        nc.vector.tensor_copy(
                            out=s_imgT[:, j * C:(j + 1) * C], in_=pt[:])
                r0 = g0 * 128
                r1 = (g0 + CH) * 128
                seg = s_imgT[:, g0 * C:(g0 + CH) * C].rearrange(
                    "p (j c) -> p j c", j=CH)
                nc.sync.dma_start(
                    out=img_pad_ap[97 + r0:97 + r1, 0:C].rearrange(
                        "(j p) c -> p j c", p=128),
                    in_=seg)
                nc.sync.dma_start(
                    out=img_pad_ap[1 + r0:1 + r1, C:2 * C].rearrange(
                        "(j p) c -> p j c", p=128),
                    in_=seg)

            # ---- phase A3: index & weight math (batched over all tiles) ----
            omT3 = omT[:].rearrange("p (t c) -> p t c", t=NT)
            dyv = omT3[:, :, 0:18:2]          # [128, NT, 9]
            dxv = omT3[:, :, 1:18:2]
            mv = omT3[:, :, 18:27]
            byx4 = byx_sb[:].rearrange("p (t k s) -> p t k s", t=NT, k=K)
            hov = byx4[:, :, :, 0]          # ho - 1 + ky  [128, NT, K]
            wov = byx4[:, :, :, 1]          # wo - 1 + kx

            def t3(name):
                t = prep.tile([128, NT * K], f32, tag=name)
                return t, t[:].rearrange("p (t k) -> p t k", t=NT)

            py, pyv = t3("py")
            px, pxv = t3("px")
            fy, fyv = t3("fy")
            fx, fxv = t3("fx")
            wy, wyv = t3("wy")
            wx, wxv = t3("wx")
            ta, tav = t3("ta")
            tb, tbv = t3("tb")
            ti = prep.tile([128, NT * K], mybir.dt.int32, tag="ti")
            tiv = ti[:].rearrange("p (t k) -> p t k", t=NT)

            V = nc.vector
            # py = dy + (ho - 1 + ky); floor & frac
            V.tensor_tensor(out=pyv, in0=dyv, in1=hov, op=Alu.add)
            V.tensor_copy(out=tiv, in_=pyv)
            V.tensor_copy(out=tav, in_=tiv)
            V.tensor_tensor(out=tbv, in0=tav, in1=pyv, op=Alu.is_gt)
            V.tensor_tensor(out=fyv, in0=tav, in1=tbv, op=Alu.subtract)
            V.tensor_tensor(out=wyv, in0=pyv, in1=fyv, op=Alu.subtract)
            # px = dx + (wo - 1 + kx)
            V.tensor_tensor(out=pxv, in0=dxv, in1=wov, op=Alu.add)
            V.tensor_copy(out=tiv, in_=pxv)
            V.tensor_copy(out=tav, in_=tiv)
            V.tensor_tensor(out=tbv, in0=tav, in1=pxv, op=Alu.is_gt)
            V.tensor_tensor(out=fxv, in0=tav, in1=tbv, op=Alu.subtract)
            V.tensor_tensor(out=wxv, in0=pxv, in1=fxv, op=Alu.subtract)

            vm0, vm0v = t3("vm0")
            vm1, vm1v = t3("vm1")
            vc0, vc0v = t3("vc0")
            vc1, vc1v = t3("vc1")
            cA, cAv = t3("cA")
            cB, cBv = t3("cB")
            # row validity (* mask) and column validity
            V.tensor_scalar(out=tav, in0=fyv, scalar1=0.0, scalar2=None, op0=Alu.is_ge)
            V.tensor_scalar(out=tbv, in0=fyv, scalar1=95.0, scalar2=None, op0=Alu.is_le)
            V.tensor_tensor(out=vm0v, in0=tav, in1=tbv, op=Alu.mult)
            V.tensor_tensor(out=vm0v, in0=vm0v, in1=mv, op=Alu.mult)
            V.tensor_scalar(out=tav, in0=fyv, scalar1=-1.0, scalar2=None, op0=Alu.is_ge)
            V.tensor_scalar(out=tbv, in0=fyv, scalar1=94.0, scalar2=None, op0=Alu.is_le)
            V.tensor_tensor(out=vm1v, in0=tav, in1=tbv, op=Alu.mult)
            V.tensor_tensor(out=vm1v, in0=vm1v, in1=mv, op=Alu.mult)
            V.tensor_scalar(out=tav, in0=fxv, scalar1=0.0, scalar2=None, op0=Alu.is_ge)
            V.tensor_scalar(out=tbv, in0=fxv, scalar1=95.0, scalar2=None, op0=Alu.is_le)
            V.tensor_tensor(out=vc0v, in0=tav, in1=tbv, op=Alu.mult)
            V.tensor_scalar(out=tav, in0=fxv, scalar1=-1.0, scalar2=None, op0=Alu.is_ge)
            V.tensor_scalar(out=tbv, in0=fxv, scalar1=94.0, scalar2=None, op0=Alu.is_le)
            V.tensor_tensor(out=vc1v, in0=tav, in1=tbv, op=Alu.mult)

            # bilinear coefficients: cy0/cy1 (carry mask), cx0/cx1
            nc.scalar.activation(out=tav, in_=wyv, func=Act.Copy, bias=1.0, scale=-1.0)
            V.tensor_tensor(out=cAv, in0=tav, in1=vm0v, op=Alu.mult)   # cy0
            V.tensor_tensor(out=cBv, in0=wyv, in1=vm1v, op=Alu.mult)   # cy1
            nc.scalar.activation(out=tav, in_=wxv, func=Act.Copy, bias=1.0, scale=-1.0)
            V.tensor_tensor(out=vc0v, in0=tav, in1=vc0v, op=Alu.mult)  # cx0
            V.tensor_tensor(out=vc1v, in0=wxv, in1=vc1v, op=Alu.mult)  # cx1

            wq = apool.tile([128, NT * K * 4], dtv)
            wq5 = wq[:].rearrange("p (t k l v) -> p t k l v", t=NT, k=K, l=2)
            V.tensor_tensor(out=wq5[:, :, :, 0, 0], in0=cAv, in1=vc0v, op=Alu.mult)
            V.tensor_tensor(out=wq5[:, :, :, 0, 1], in0=cBv, in1=vc0v, op=Alu.mult)
            V.tensor_tensor(out=wq5[:, :, :, 1, 0], in0=cAv, in1=vc1v, op=Alu.mult)
            V.tensor_tensor(out=wq5[:, :, :, 1, 1], in0=cBv, in1=vc1v, op=Alu.mult)

            # quad-gather indices: clamp(96*fy + fx + 97, 0, 9312), f32
            idxf = prep.tile([128, NT * K], f32)
            idxfv = idxf[:].rearrange("p (t k) -> p t k", t=NT)
            V.scalar_tensor_tensor(out=idxfv, in0=fyv, scalar=96.0, in1=fxv,
                                   op0=Alu.mult, op1=Alu.add)
            V.tensor_scalar(out=idxf[:], in0=idxf[:], scalar1=97.0,
                            scalar2=None, op0=Alu.add)
            V.tensor_scalar(out=idxf[:], in0=idxf[:], scalar1=0.0, scalar2=9312.0,
                            op0=Alu.max, op1=Alu.min)

            # ---- wrapped SWDGE index layout: idx16[p%16, k*288 + t*8 + p//16]
            # (replicated x8 across 16-partition groups) ----
            idx16 = apool.tile([128, K * NT * 8], i16)
            idx16v = idx16[:].rearrange("p (k t q) -> p k t q", k=K, t=NT)
            for p1 in range(8):
                ps = sel_ps.tile([16, NT * K], f32, tag="sel")
                nc.tensor.matmul(
                    out=ps[:], lhsT=ident[:, 16 * p1:16 * (p1 + 1)],
                    rhs=idxf[:], start=True, stop=True)
                psv = ps[:].rearrange("p (t k) -> p t k", t=NT)
                V.tensor_copy(
                    out=idx16v[0:16, :, :, p1].rearrange("p k t -> p t k"),
                    in_=psv)
            for g_ in range(1, 8):
                nc.sync.dma_start(out=idx16[16 * g_:16 * (g_ + 1), :],
                                  in_=idx16[0:16, :])

            prep_ctx.close()
            gpool = ctx.enter_context(tc.tile_pool(name="gather", bufs=2))
            spool = ctx.enter_context(tc.tile_pool(name="samp", bufs=2))
            stpool = ctx.enter_context(tc.tile_pool(name="sampT", bufs=3))
            obpool = ctx.enter_context(tc.tile_pool(name="ob", bufs=3))

            # ---- phase B: per-(group, tap) gather -> bilinear; per-tile
            # transpose -> matmul ----
            wq5v = wq[:].rearrange("p (t k l v) -> p t k l v", t=NT, k=K, l=2)
            w2v = w2_sb[:].rearrange("p (f c) -> p f c", f=5)
            for t0, gt in GROUPS:
                gk = gpool.tile([128, gt * K * 256], dtv, tag="gk")
                gk4 = gk[:].rearrange("p (k t e) -> p k t e", k=K, t=gt)
                for k in range(K):
                    nc.gpsimd.dma_gather(
                        out_ap=gk4[:, k, :, :],
                        in_ap=img_pad_ov,
                        idxs_ap=idx16v[:, k, t0:t0 + gt, :].rearrange(
                            "p t q -> p (t q)"),
                        num_idxs=gt * 128,
                        num_idxs_reg=gt * 128,
                        elem_size=256,
                        elem_step=128,
                        queue_num=k % 4,
                    )
                samp = spool.tile([128, gt * K * C], dtv, tag="samp")
                samp4 = samp[:].rearrange("p (t k c) -> p t k c", t=gt, k=K)
                for k in range(K):
                    g5 = gk4[:, k, :, :].rearrange(
                        "p t (l v c) -> p t l v c", l=2, v=2)
                    wq_b = wq5v[:, t0:t0 + gt, k, :, :].unsqueeze(4).to_broadcast(
                        [128, gt, 2, 2, C])
                    wg = spool.tile([128, gt * 4 * C], dtv, tag="wg")
                    wg5 = wg[:].rearrange("p (t l v c) -> p t l v c",
                                          t=gt, l=2, v=2)
                    V.tensor_tensor(out=wg5, in0=g5, in1=wq_b, op=Alu.mult)
                    s01 = spool.tile([128, gt * 2 * C], dtv, tag="s01")
                    s013 = s01[:].rearrange("p (t v c) -> p t v c", t=gt, v=2)
                    V.tensor_tensor(out=s013, in0=wg5[:, :, 0, :, :],
                                    in1=wg5[:, :, 1, :, :], op=Alu.add)
                    V.tensor_tensor(out=samp4[:, :, k, :], in0=s013[:, :, 0, :],
                                    in1=s013[:, :, 1, :], op=Alu.add)

                samp_fl = samp[:].rearrange("p (t x) -> p t x", t=gt)
                for tl in range(gt):
                    sampT = stpool.tile([128, 5 * 128], dtv)
                    for ci, off in enumerate(CHUNK_OFFS):
                        pt = tb_ps.tile([128, 128], dtv, tag="trB")
                        nc.tensor.transpose(
                            out=pt[:], in_=samp_fl[:, tl, off:off + 128],
                            identity=identB[:])
                        nc.scalar.activation(
                            out=sampT[:, ci * 128:(ci + 1) * 128], in_=pt[:],
                            func=Act.Copy)

                    po = opsum.tile([Co, 128], f32)
                    for ci in range(5):
                        nc.tensor.matmul(
                            out=po[:], lhsT=w2v[:, ci, :],
                            rhs=sampT[:, ci * 128:(ci + 1) * 128],
                            start=(ci == 0), stop=(ci == 4))

                    t = t0 + tl
                    ob = obpool.tile([Co, 128], f32)
                    V.tensor_scalar(out=ob[:], in0=po[:], scalar1=bias_sb[:, 0:1],
                                    scalar2=None, op0=Alu.add)
                    nc.sync.dma_start(out=out_ap[:, t * 128:(t + 1) * 128],
                                      in_=ob[:])

    nc.compile()
    nc.m = get_hw_module(nc.m)
    return nc


def _host_prep(input, offset, mask, weight, bias):
    import ml_dtypes
    f32 = np.float32
    input = np.ascontiguousarray(input, dtype=f32)
    offset = np.ascontiguousarray(offset, dtype=f32)
    mask = np.ascontiguousarray(mask, dtype=f32)
    weight = np.ascontiguousarray(weight, dtype=f32)
    bias = np.ascontiguousarray(bias, dtype=f32)

    # weight [Co, C, 3, 3] -> W2r[(t*64+c), co], chunked at CHUNK_OFFS with
    # the 448-overlap region zeroed out of chunk 4 (rows 448..511 live in
    # chunk 3).
    wr = weight.reshape(Co, C, K)                     # [co, c, t]
    W2r = np.transpose(wr, (2, 1, 0)).reshape(C * K, Co)  # [(t,c), co]
    w2 = np.zeros((5, 128, Co), dtype=f32)
    w2[0] = W2r[0:128]
    w2[1] = W2r[128:256]
    w2[2] = W2r[256:384]
    w2[3] = W2r[384:512]
    w2[4, 64:128] = W2r[512:576]
    w2 = w2.reshape(5 * 128, Co)
    if DT16:
        w2 = w2.astype(ml_dtypes.bfloat16)

    biasv = bias.reshape(Co, 1)
    kyv = (np.arange(K, dtype=f32) // 3)
    kxv = (np.arange(K, dtype=f32) % 3)

    pix = np.arange(NPIX).reshape(NT, 128)
    in_maps = []
    for core in range(N_CORES):
        b, h = core // 2, core % 2
        ho0 = h * HHALF
        ho = ho0 + pix // W
        wo = pix % W
        base_y = (ho - 1)[:, :, None] + kyv[None, None, :]   # [NT, 128, K]
        base_x = (wo - 1)[:, :, None] + kxv[None, None, :]
        byx = np.stack([base_y, base_x], axis=-1)            # [NT, 128, K, 2]
        byx = np.ascontiguousarray(
            byx.transpose(1, 0, 2, 3).reshape(128, NT * K * 2), dtype=f32)
        offmask = np.concatenate(
            [offset[b, :, ho0:ho0 + HHALF, :].reshape(18, NPIX),
             mask[b, :, ho0:ho0 + HHALF, :].reshape(K, NPIX)], axis=0)
        in_maps.append({
            "img": input[b].reshape(C, HW),
            "offmask": np.ascontiguousarray(offmask),
            "byx": byx,
            "w2": w2,
            "biasv": biasv,
        })
    return in_maps


def kernel(input, offset, mask, weight, bias):
    from concourse.bass_utils import run_bass_kernel_spmd

    if "nc" not in _CACHE:
        _CACHE["nc"] = _build_module()
    nc = _CACHE["nc"]

    in_maps = _host_prep(input, offset, mask, weight, bias)
    res = run_bass_kernel_spmd(nc, in_maps, core_ids=list(range(N_CORES)))

    out = np.empty((B, Co, H, W), dtype=np.float32)
    for core in range(N_CORES):
        b, h = core // 2, core % 2
        ho0 = h * HHALF
        out[b, :, ho0:ho0 + HHALF, :] = \
            res.results[core]["out"].reshape(Co, HHALF, W)
    return out
